# revision 21
# baseline (speedup 1.0000x reference)
"""Trainium2 Bass kernel for nn_MixtureOfRanksLayer (moe_routing).

Strategy: data-parallel over the token axis (N=4096 -> 512 tokens/core on 8
cores), all 8 experts evaluated densely per core (weighted by the top-2
routing mask, which matches the reference math exactly).  No collectives:
the full output is just the concatenation of per-core token slices.

Per-core pipeline (tok = 512 on the matmul free dim throughout):
  xT   = x.T                         (PE transpose, 16x4 128x128 tiles)
  T1T  = U1cat.T @ xT                [er=512, tok]   K=2048, fp32r
  lg   = x @ gate_w.T + gate_b       [tok, 8]        via xT as lhsT
  w    = top2-renormalized weights   (masked-max + sigmoid(l1-l2), exact)
  per expert pair (2c, 2c+1):
    h0/h1 = relu(v1.T @ T1T + b1)    row-tiled concurrent K=64 matmuls
    T2T  += u2.T-chunks @ hT         col-tiled concurrent pairs, bf16
  T2T *= w (broadcast via SEL matmul), OUT = T2T.T @ V2cat + w.T @ b2

Walrus limits fp32/fp32r Matmult to ONE sync wait, so evacuation engines
are fixed per tile class and tiny [1,1] "absorber" matmuls consume each
DMA-completion wait before the real matmuls need the data.
"""

from contextlib import ExitStack

import numpy as np

import concourse.bass as bass
import concourse.bacc as bacc
import concourse.mybir as mybir
import concourse.tile as tile
from concourse.tile_rust import add_dep_helper

dt = mybir.dt
AF = mybir.ActivationFunctionType
ALU = mybir.AluOpType
AX = mybir.AxisListType

# Model dims (hardcoded for this problem)
E, D, H, R = 8, 2048, 8192, 64
N_TOK = 4096
NCORES = 8

FULL_CFG = dict(E=E, D=D, H=H, R=R, NT=N_TOK // NCORES)

# fraction of relu/evac chunks handled by ScalarE (rest on VectorE)
ACT_OF_16 = 9


def build(cfg):
    """Build the single-core Bass module (SPMD: same NEFF on all cores)."""
    E, D, H, R, NT = cfg["E"], cfg["D"], cfg["H"], cfg["R"], cfg["NT"]
    TOKC = NT // 128          # token chunks of 128
    DC = D // 128             # contraction chunks over d_model
    HC = H // 128             # hidden chunks of 128
    ER = E * R                # stacked expert-rank axis (512)
    ERC = E // 2              # expert pairs
    DD = D // 512             # output free-dim chunks
    f32 = dt.float32
    f32r = dt.float32r
    bf16 = dt.bfloat16

    nc = bacc.Bacc("TRN2", debug=False)

    x_d = nc.dram_tensor("x", [NT, D], f32, kind="ExternalInput").ap()
    u1c_d = nc.dram_tensor("u1c", [D, ER], f32r, kind="ExternalInput").ap()
    v1_d = nc.dram_tensor("v1", [E, R, H], f32r, kind="ExternalInput").ap()
    b1r_d = nc.dram_tensor("b1r", [128, E * HC], f32, kind="ExternalInput").ap()
    u2r_d = nc.dram_tensor("u2r", [E, 128, HC, R], f32, kind="ExternalInput").ap()
    v2c_d = nc.dram_tensor("v2c", [ER, D], f32r, kind="ExternalInput").ap()
    b2_d = nc.dram_tensor("b2", [E, D], f32r, kind="ExternalInput").ap()
    gw_d = nc.dram_tensor("gw", [E, D], f32, kind="ExternalInput").ap()
    gb_d = nc.dram_tensor("gb", [1, E], f32r, kind="ExternalInput").ap()
    onesr_d = nc.dram_tensor("onesr", [1, 128], f32r, kind="ExternalInput").ap()
    ident_d = nc.dram_tensor("ident", [128, 128], f32, kind="ExternalInput").ap()
    sel_d = nc.dram_tensor("sel", [E, ERC, 128], f32r, kind="ExternalInput").ap()
    out_d = nc.dram_tensor("out", [NT, D], f32, kind="ExternalOutput").ap()

    with ExitStack() as ctx:
        tc = ctx.enter_context(tile.TileContext(nc))

        const = ctx.enter_context(tc.tile_pool(name="const", bufs=1))
        persist = ctx.enter_context(tc.tile_pool(name="persist", bufs=1))

        ident = const.tile([128, 128], f32, tag="ident")
        nc.sync.dma_start(ident, ident_d)
        sel_sb = const.tile([E, ERC, 128], f32r, tag="sel")
        nc.sync.dma_start(sel_sb, sel_d)
        ones_sb = const.tile([1, 128], f32r, tag="ones")
        nc.sync.dma_start(ones_sb, onesr_d)
        one_bf = const.tile([1, 1], bf16, tag="one_bf")
        nc.vector.memset(one_bf, 1.0)
        gb_sb = const.tile([1, E], f32r, tag="gb")
        nc.sync.dma_start(gb_sb, gb_d)
        b2_sb = const.tile([E, D], f32r, tag="b2")
        nc.sync.dma_start(b2_sb, b2_d)
        b1r_sb = const.tile([128, E * HC], f32, tag="b1r")
        nc.sync.dma_start(b1r_sb, b1r_d)
        gw_sb = const.tile([E, D], f32, tag="gw")
        nc.sync.dma_start(gw_sb, gw_d)

        gwT = persist.tile([128, DC, E], f32r, tag="gwT")
        T1Tp = [persist.tile([128, NT], f32r, tag=f"t1t{c}", name=f"t1t{c}")
                for c in range(ERC)]
        wT = persist.tile([E, NT], f32r, tag="wT")
        Wbc = [persist.tile([128, NT], f32, tag=f"wbc{c}", name=f"wbc{c}")
               for c in range(ERC)]
        T2Ts = [persist.tile([128, NT], f32r, tag=f"t2t{c}", name=f"t2t{c}")
                for c in range(ERC)]
        V2sb = [persist.tile([128, D], f32r, tag=f"v2{c}", name=f"v2{c}")
                for c in range(ERC)]

        # A single persistent [1,1] PSUM sink: every DMA-wait absorber matmul
        # writes here.  Same-engine WAW deps need no semaphores, so each
        # absorber carries exactly one wait (the DMA completion).
        snk = ctx.enter_context(tc.tile_pool(name="snk", bufs=1, space="PSUM"))
        sink = snk.tile([1, 1], f32, tag="sink")

        def absorb(src_ap):
            # [1,1] bf16 garbage-read matmul: makes PE observe src's writer
            # (f32r matmuls at size 1 are invalid ISA, so view as bf16)
            bap = src_ap.bitcast(bf16)[0:1, 0:1]
            return nc.tensor.matmul(sink, lhsT=bap, rhs=bap,
                                    start=True, stop=True, skip_group_check=True)

        def absorb_pre(src_ap, one_ap, dty):
            return nc.tensor.transpose(sink.bitcast(dty)[0:1, 0:1], src_ap, one_ap)

        # ---------------- Phase 1: xT, T1T, gating ----------------
        with ExitStack() as s1:
            p1 = s1.enter_context(tc.tile_pool(name="p1", bufs=1))
            sm = s1.enter_context(tc.tile_pool(name="sm", bufs=2))
            ps_tp = s1.enter_context(tc.tile_pool(name="ps_tp", bufs=2, space="PSUM"))
            ps_t1 = s1.enter_context(tc.tile_pool(name="ps_t1", bufs=2, space="PSUM"))
            ps_lg = s1.enter_context(tc.tile_pool(name="ps_lg", bufs=1, space="PSUM"))
            ps_wb = s1.enter_context(tc.tile_pool(name="ps_wb", bufs=2, space="PSUM"))

            x_sb = p1.tile([128, TOKC, D], f32, tag="x")
            nc.sync.dma_start(x_sb, x_d.rearrange("(t p) d -> p t d", p=128))
            U1sb = p1.tile([128, DC, ER], f32r, tag="u1")
            nc.sync.dma_start(U1sb, u1c_d.rearrange("(dc p) er -> p dc er", p=128))
            xT = p1.tile([128, DC, NT], f32r, tag="xT")

            absorb(ident[0:1, 0:1])
            absorb(gw_sb[0:1, 0:1])
            absorb(x_sb[0:1, 0, 0:1])

            # gate_w.T tiles via PE transpose (evac: DVE)
            for dc in range(DC):
                pst = ps_tp.tile([128, 128], f32, tag="tp")
                nc.tensor.transpose(pst[:, 0:E], gw_sb[:, dc * 128:(dc + 1) * 128],
                                    ident[0:E, 0:E])
                nc.vector.tensor_copy(gwT[:, dc, :], pst[:, 0:E])

            # x.T tiles via PE transpose (evac: DVE)
            for dc in range(DC):
                for t in range(TOKC):
                    pst = ps_tp.tile([128, 128], f32, tag="tp")
                    nc.tensor.transpose(pst, x_sb[:, t, dc * 128:(dc + 1) * 128], ident)
                    nc.vector.tensor_copy(xT[:, dc, t * 128:(t + 1) * 128], pst)

            absorb(U1sb[0:1, 0, 0:1])

            # T1T = U1.T @ xT   [er, tok], accumulated over DC (evac: ACT)
            for c in range(ERC):
                pt = ps_t1.tile([128, NT], f32, tag="t1")
                for dc in range(DC):
                    nc.tensor.matmul(pt,
                                     lhsT=U1sb[:, dc, c * 128:(c + 1) * 128],
                                     rhs=xT[:, dc, :],
                                     start=(dc == 0), stop=(dc == DC - 1))
                nc.vector.tensor_copy(T1Tp[c], pt)

            absorb(gb_sb[0:1, 0:1])
            absorb(ones_sb[0:1, 0:1])

            # logits, top-2 renormalized weights
            for t in range(TOKC):
                pl = ps_lg.tile([128, E], f32, tag="lg")
                for dc in range(DC):
                    nc.tensor.matmul(pl,
                                     lhsT=xT[:, dc, t * 128:(t + 1) * 128],
                                     rhs=gwT[:, dc, :],
                                     start=(dc == 0), stop=False)
                nc.tensor.matmul(pl, lhsT=ones_sb, rhs=gb_sb,
                                 start=False, stop=True)
                lg = sm.tile([128, E], f32, tag="lg_sb")
                nc.vector.tensor_copy(lg, pl)
                l1 = sm.tile([128, 1], f32, tag="l1")
                nc.vector.reduce_max(out=l1, in_=lg, axis=AX.X)
                m1t = sm.tile([128, E], f32, tag="m1t")
                nc.vector.tensor_scalar(m1t, lg, l1, None, op0=ALU.is_equal)
                lm = sm.tile([128, E], f32, tag="lm")
                nc.vector.tensor_scalar(lm, m1t, -1e30, None, op0=ALU.mult)
                nc.vector.tensor_add(lm, lm, lg)
                l2 = sm.tile([128, 1], f32, tag="l2")
                nc.vector.reduce_max(out=l2, in_=lm, axis=AX.X)
                m2t = sm.tile([128, E], f32, tag="m2t")
                nc.vector.tensor_scalar(m2t, lm, l2, None, op0=ALU.is_equal)
                dif = sm.tile([128, 1], f32, tag="dif")
                nc.vector.tensor_sub(dif, l1, l2)
                s1v = sm.tile([128, 1], f32, tag="s1v")
                nc.scalar.activation(s1v, dif, AF.Sigmoid)
                s0v = sm.tile([128, 1], f32, tag="s0v")
                nc.scalar.activation(s0v, dif, AF.Sigmoid, scale=-1.0)
                wa = sm.tile([128, E], f32, tag="wa")
                nc.vector.tensor_scalar(wa, m1t, s1v, None, op0=ALU.mult)
                wb_ = sm.tile([128, E], f32, tag="wb_")
                nc.vector.tensor_scalar(wb_, m2t, s0v, None, op0=ALU.mult)
                w_sb = sm.tile([128, E], f32, tag="w_sb")
                nc.vector.tensor_add(w_sb, wa, wb_)
                pw = ps_tp.tile([128, 128], f32, tag="tp")
                nc.tensor.transpose(pw[0:E, :], w_sb, ident)
                nc.vector.tensor_copy(wT[:, t * 128:(t + 1) * 128], pw[0:E, :])

            absorb(sel_sb[0:1, 0, 0:1])

            # broadcast per-expert weights across partitions: SEL.T @ wT
            for c in range(ERC):
                pb = ps_wb.tile([128, NT], f32, tag="wb")
                nc.tensor.matmul(pb, lhsT=sel_sb[:, c, :], rhs=wT,
                                 start=True, stop=True)
                nc.vector.tensor_copy(Wbc[c], pb)

            # engine-tick observers: bring PE's view of the ACT/DVE clocks
            # current so later pool-alloc deps reduce to single waits
            obs_a = persist.tile([1, 1], f32, tag="obs_a")
            nc.scalar.copy(obs_a, b1r_sb[0:1, 0:1])
            absorb(obs_a)
            obs_late = absorb(Wbc[ERC - 1][0:1, 0:1])

        # ---------------- Phase 2: experts ----------------
        p2v = ctx.enter_context(tc.tile_pool(name="p2v", bufs=2))
        p2u = ctx.enter_context(tc.tile_pool(name="p2u", bufs=2))
        p2h = ctx.enter_context(tc.tile_pool(name="p2h", bufs=4))
        s2 = ExitStack()
        ps_h0 = s2.enter_context(tc.tile_pool(name="ps_h0", bufs=2, space="PSUM"))
        ps_h1 = s2.enter_context(tc.tile_pool(name="ps_h1", bufs=2, space="PSUM"))
        ps_t2 = s2.enter_context(tc.tile_pool(name="ps_t2", bufs=1, space="PSUM"))

        relu_i = 0

        def relu_evac(dst, src, bias_ap, force_dve=False):
            nonlocal relu_i
            relu_i += 1
            if not force_dve and relu_i % 16 < ACT_OF_16:
                nc.scalar.activation(dst, src, AF.Relu, bias=bias_ap)
            else:
                nc.vector.tensor_scalar(dst, src, bias_ap, 0.0,
                                        op0=ALU.add, op1=ALU.max)

        for c in range(ERC):
            e0, e1 = 2 * c, 2 * c + 1
            v1p = p2v.tile([128, H], f32r, tag="v1")
            absorb_pre(v1p.bitcast(bf16)[0:1, 0:1], one_bf, bf16)
            nc.sync.dma_start(
                v1p, v1_d[e0:e1 + 1].rearrange("two r h -> (two r) h"))
            u2p = p2u.tile([128, 2, HC, R], bf16, tag="u2")
            absorb_pre(u2p[0:1, 0, 0, 0:1], one_bf, bf16)
            nc.gpsimd.dma_start(  # fp32 -> bf16 cast DMA
                u2p, u2r_d[e0:e1 + 1].rearrange("two p hc r -> p two hc r"))

            if c == 1:  # phase-3 loads, emitted late so they don't hog queues
                for cc in range(ERC):
                    nc.sync.dma_start(V2sb[cc], v2c_d[cc * 128:(cc + 1) * 128, :])

            absorb(v1p[0:1, 0:1])
            absorb(u2p[0:1, 0, 0, 0:1])

            # separate PSUM banks per col-tile half (start=True is per-bank)
            pt2a = ps_t2.tile([128, NT], f32, tag="t2a", name="pt2a")
            pt2b = ps_t2.tile([128, NT], f32, tag="t2b", name="pt2b")
            if c == 0:
                # first PE touch of the ps_t2 pool soaks its alloc wait;
                # scheduling-only edge keeps it after the clock observers
                for pz in (pt2a, pt2b):
                    dmy = nc.tensor.matmul(pz[0:1, 0:1], lhsT=ident[0:1, 0:1],
                                           rhs=ident[0:1, 0:1], start=True,
                                           stop=True, skip_group_check=True)
                    add_dep_helper(dmy.ins, obs_late.ins, sync=False,
                                   reason="order pt2 pre-touch after observers")
            for hc in range(HC):
                ph0 = ps_h0.tile([128, NT], f32, tag="h0", name="ph0")
                ph1 = ps_h1.tile([128, NT], f32, tag="h1", name="ph1")
                # m2: row-tiled concurrent K=64 matmuls (one per expert)
                nc.tensor.matmul(ph0,
                                 lhsT=v1p[0:64, hc * 128:(hc + 1) * 128],
                                 rhs=T1Tp[c][0:64, :],
                                 start=True, stop=True)
                nc.tensor.matmul(ph1,
                                 lhsT=v1p[64:128, hc * 128:(hc + 1) * 128],
                                 rhs=T1Tp[c][64:128, :],
                                 start=True, stop=True)
                hT0 = p2h.tile([128, NT], bf16, tag="h0", name="hT0")
                hT1 = p2h.tile([128, NT], bf16, tag="h1", name="hT1")
                if c == 0 and hc == 0:
                    absorb_pre(hT0[0:1, 0:1], one_bf, bf16)
                    absorb_pre(hT1[0:1, 0:1], one_bf, bf16)
                relu_evac(hT0, ph0, b1r_sb[:, e0 * HC + hc:e0 * HC + hc + 1],
                          force_dve=(hc == 0))
                relu_evac(hT1, ph1, b1r_sb[:, e1 * HC + hc:e1 * HC + hc + 1],
                          force_dve=(hc == 0))
                # m3: col-tiled concurrent pair, bf16, accumulate over hc
                nc.tensor.matmul(pt2a[0:64, :], lhsT=u2p[:, 0, hc, :], rhs=hT0,
                                 start=(hc == 0), stop=(hc == HC - 1),
                                 skip_group_check=True)
                nc.tensor.matmul(pt2b[64:128, :], lhsT=u2p[:, 1, hc, :], rhs=hT1,
                                 start=(hc == 0), stop=(hc == HC - 1),
                                 skip_group_check=True)
            nc.vector.tensor_tensor(T2Ts[c][0:64, :], pt2a[0:64, :],
                                    Wbc[c][0:64, :], op=ALU.mult)
            nc.vector.tensor_tensor(T2Ts[c][64:128, :], pt2b[64:128, :],
                                    Wbc[c][64:128, :], op=ALU.mult)

        s2.close()

        # ---------------- Phase 3: combine ----------------
        absorb(b2_sb[0:1, 0:1])
        for c in range(ERC):
            absorb(V2sb[c][0:1, 0:1])
        absorb(T2Ts[ERC - 1][0:1, 0:1])
        ps_o = ctx.enter_context(tc.tile_pool(name="ps_o", bufs=2, space="PSUM"))
        p3o = ctx.enter_context(tc.tile_pool(name="p3o", bufs=4))
        for t in range(TOKC):
            for dd in range(DD):
                po = ps_o.tile([128, 512], f32, tag="o", name="po")
                for c in range(ERC):
                    nc.tensor.matmul(po,
                                     lhsT=T2Ts[c][:, t * 128:(t + 1) * 128],
                                     rhs=V2sb[c][:, dd * 512:(dd + 1) * 512],
                                     start=(c == 0), stop=False)
                nc.tensor.matmul(po, lhsT=wT[:, t * 128:(t + 1) * 128],
                                 rhs=b2_sb[:, dd * 512:(dd + 1) * 512],
                                 start=False, stop=True)
                ob = p3o.tile([128, 512], f32, tag="ob")
                nc.scalar.copy(ob, po)
                nc.sync.dma_start(
                    out_d[t * 128:(t + 1) * 128, dd * 512:(dd + 1) * 512], ob)

    nc.compile()
    return nc


def build_tiny():
    """Trivial NEFF for measuring dispatch overhead."""
    from contextlib import ExitStack as _ES
    nc = bacc.Bacc("TRN2", debug=False)
    a_d = nc.dram_tensor("a", [1, 8], dt.float32, kind="ExternalInput").ap()
    o_d = nc.dram_tensor("o", [1, 8], dt.float32, kind="ExternalOutput").ap()
    with _ES() as ctx:
        tc = ctx.enter_context(tile.TileContext(nc))
        p = ctx.enter_context(tc.tile_pool(name="p", bufs=1))
        t = p.tile([1, 8], dt.float32, tag="t")
        nc.sync.dma_start(t, a_d)
        nc.sync.dma_start(o_d, t)
    nc.compile()
    return nc


def prep_inputs(x, u1, v1, b1, u2, v2, b2, gate_w, gate_b, cfg):
    """Host-side layout prep (pure reshapes/transposes) + per-core sharding."""
    E, D, H, R, NT = cfg["E"], cfg["D"], cfg["H"], cfg["R"], cfg["NT"]
    HC = H // 128
    ER = E * R
    ERC = E // 2
    f = lambda a: np.ascontiguousarray(np.asarray(a, dtype=np.float32))

    x = f(x)
    u1c = f(np.asarray(u1, np.float32).transpose(1, 0, 2).reshape(D, ER))
    v1 = f(v1)
    # b1r[p, e*HC+hc] = b1[e, hc*128+p]  (per-partition bias columns)
    b1r = f(np.asarray(b1, np.float32).reshape(E, HC, 128).transpose(2, 0, 1)
            .reshape(128, E * HC))
    # u2r[e, p, hc, r] = u2[e, hc*128+p, r]
    u2r = f(np.asarray(u2, np.float32).reshape(E, HC, 128, R).transpose(0, 2, 1, 3))
    v2c = f(np.asarray(v2, np.float32).reshape(ER, D))
    b2 = f(b2)
    gw = f(gate_w)
    gb = f(np.asarray(gate_b, np.float32).reshape(1, E))
    ident = np.eye(128, dtype=np.float32)
    onesr = np.ones((1, 128), dtype=np.float32)
    sel = np.zeros((E, ERC, 128), dtype=np.float32)
    for c in range(ERC):
        sel[2 * c, c, 0:64] = 1.0
        sel[2 * c + 1, c, 64:128] = 1.0

    ncores = x.shape[0] // NT
    shared = dict(u1c=u1c, v1=v1, b1r=b1r, u2r=u2r, v2c=v2c, b2=b2,
                  gw=gw, gb=gb, ident=ident, sel=sel, onesr=onesr)
    in_maps = []
    for c in range(ncores):
        m = dict(shared)
        m["x"] = np.ascontiguousarray(x[c * NT:(c + 1) * NT])
        in_maps.append(m)
    return in_maps


_BUILT = {}


def _get_nc(cfg_key=None):
    if cfg_key is None:
        cfg_key = "full"
    if cfg_key not in _BUILT:
        _BUILT[cfg_key] = build(FULL_CFG)
    return _BUILT[cfg_key]


def run(inputs, trace=False):
    """Run on 8 cores; returns (full_output, BassKernelResults)."""
    import concourse.bass_utils as bass_utils

    nc = _get_nc()
    in_maps = prep_inputs(**inputs, cfg=FULL_CFG)
    res = bass_utils.run_bass_kernel_spmd(
        nc, in_maps, core_ids=list(range(len(in_maps))), trace=trace)
    out = np.concatenate([r["out"] for r in res.results], axis=0)
    return out, res


def kernel(**inputs) -> np.ndarray:
    out, _ = run(inputs, trace=False)
    return out


if __name__ == "__main__":
    # smoke-build only
    nc = _get_nc()
    print("built ok:", nc)


# revision 25
# speedup vs baseline: 77.7127x; 77.7127x over previous
"""Trainium2 Bass kernel for nn_MixtureOfRanksLayer (moe_routing).

Strategy: data-parallel over the token axis (N=4096 -> 512 tokens/core on 8
cores), all 8 experts evaluated densely per core (weighted by the top-2
routing mask, which matches the reference math exactly).  No collectives:
the full output is just the concatenation of per-core token slices.

Per-core pipeline (tok = 512 on the matmul free dim throughout):
  xT   = x.T                         (PE transpose, 16x4 128x128 tiles)
  T1T  = U1cat.T @ xT                [er=512, tok]   K=2048, fp32r
  lg   = x @ gate_w.T + gate_b       [tok, 8]        via xT as lhsT
  w    = top2-renormalized weights   (masked-max + sigmoid(l1-l2), exact)
  per expert pair (2c, 2c+1):
    h0/h1 = relu(v1.T @ T1T + b1)    row-tiled concurrent K=64 matmuls
    T2T  += u2.T-chunks @ hT         col-tiled concurrent pairs, bf16
  T2T *= w (broadcast via SEL matmul), OUT = T2T.T @ V2cat + w.T @ b2

Walrus limits fp32/fp32r Matmult to ONE sync wait, so evacuation engines
are fixed per tile class and tiny [1,1] "absorber" matmuls consume each
DMA-completion wait before the real matmuls need the data.
"""

from contextlib import ExitStack

import numpy as np

import concourse.bass as bass
import concourse.bacc as bacc
import concourse.mybir as mybir
import concourse.tile as tile
from concourse.tile_rust import add_dep_helper

dt = mybir.dt
AF = mybir.ActivationFunctionType
ALU = mybir.AluOpType
AX = mybir.AxisListType

# Model dims (hardcoded for this problem)
E, D, H, R = 8, 2048, 8192, 64
N_TOK = 4096
NCORES = 8

FULL_CFG = dict(E=E, D=D, H=H, R=R, NT=N_TOK // NCORES)

# fraction of relu/evac chunks handled by ScalarE (rest on VectorE)
ACT_OF_16 = 10


def build(cfg, rep=1):
    """Build the single-core Bass module (SPMD: same NEFF on all cores)."""
    E, D, H, R, NT = cfg["E"], cfg["D"], cfg["H"], cfg["R"], cfg["NT"]
    TOKC = NT // 128          # token chunks of 128
    DC = D // 128             # contraction chunks over d_model
    HC = H // 128             # hidden chunks of 128
    ER = E * R                # stacked expert-rank axis (512)
    ERC = E // 2              # expert pairs
    DD = D // 512             # output free-dim chunks
    f32 = dt.float32
    f32r = dt.float32r
    bf16 = dt.bfloat16

    nc = bacc.Bacc("TRN2", debug=False)

    x_d = nc.dram_tensor("x", [NT, D], f32, kind="ExternalInput").ap()
    u1c_d = nc.dram_tensor("u1c", [D, ER], f32r, kind="ExternalInput").ap()
    v1_d = nc.dram_tensor("v1", [E, R, H], f32r, kind="ExternalInput").ap()
    b1r_d = nc.dram_tensor("b1r", [128, E * HC], f32, kind="ExternalInput").ap()
    u2r_d = nc.dram_tensor("u2r", [E, 128, HC, R], bf16, kind="ExternalInput").ap()
    v2c_d = nc.dram_tensor("v2c", [ER, D], f32r, kind="ExternalInput").ap()
    b2_d = nc.dram_tensor("b2", [E, D], f32r, kind="ExternalInput").ap()
    gw_d = nc.dram_tensor("gw", [E, D], f32, kind="ExternalInput").ap()
    gb_d = nc.dram_tensor("gb", [1, E], f32r, kind="ExternalInput").ap()
    onesr_d = nc.dram_tensor("onesr", [1, 128], f32r, kind="ExternalInput").ap()
    ident_d = nc.dram_tensor("ident", [128, 128], f32, kind="ExternalInput").ap()
    sel_d = nc.dram_tensor("sel", [E, ERC, 128], f32r, kind="ExternalInput").ap()
    out_d = nc.dram_tensor("out", [NT, D], f32, kind="ExternalOutput").ap()

    with ExitStack() as ctx:
        tc = ctx.enter_context(tile.TileContext(nc))

        const = ctx.enter_context(tc.tile_pool(name="const", bufs=1))
        persist = ctx.enter_context(tc.tile_pool(name="persist", bufs=1))

        ident = const.tile([128, 128], f32, tag="ident")
        nc.sync.dma_start(ident, ident_d)
        sel_sb = const.tile([E, ERC, 128], f32r, tag="sel")
        nc.sync.dma_start(sel_sb, sel_d)
        ones_sb = const.tile([1, 128], f32r, tag="ones")
        nc.sync.dma_start(ones_sb, onesr_d)
        one_bf = const.tile([1, 1], bf16, tag="one_bf")
        nc.vector.memset(one_bf, 1.0)
        gb_sb = const.tile([1, E], f32r, tag="gb")
        nc.sync.dma_start(gb_sb, gb_d)
        b2_sb = const.tile([E, D], f32r, tag="b2")
        nc.sync.dma_start(b2_sb, b2_d)
        b1r_sb = const.tile([128, E * HC], f32, tag="b1r")
        nc.sync.dma_start(b1r_sb, b1r_d)
        gw_sb = const.tile([E, D], f32, tag="gw")
        nc.sync.dma_start(gw_sb, gw_d)

        gwT = persist.tile([128, DC, E], f32r, tag="gwT")
        T1Tp = [persist.tile([128, NT], f32r, tag=f"t1t{c}", name=f"t1t{c}")
                for c in range(ERC)]
        wT = persist.tile([E, NT], f32r, tag="wT")
        Wbc = [persist.tile([128, NT], f32, tag=f"wbc{c}", name=f"wbc{c}")
               for c in range(ERC)]
        T2Ts = [persist.tile([128, NT], f32r, tag=f"t2t{c}", name=f"t2t{c}")
                for c in range(ERC)]
        V2sb = [persist.tile([128, D], f32r, tag=f"v2{c}", name=f"v2{c}")
                for c in range(ERC)]

        # A single persistent [1,1] PSUM sink: every DMA-wait absorber matmul
        # writes here.  Same-engine WAW deps need no semaphores, so each
        # absorber carries exactly one wait (the DMA completion).
        snk = ctx.enter_context(tc.tile_pool(name="snk", bufs=1, space="PSUM"))
        sink = snk.tile([1, 1], f32, tag="sink")

        def absorb(src_ap):
            # [1,1] bf16 garbage-read matmul: makes PE observe src's writer
            # (f32r matmuls at size 1 are invalid ISA, so view as bf16)
            bap = src_ap.bitcast(bf16)[0:1, 0:1]
            return nc.tensor.matmul(sink, lhsT=bap, rhs=bap,
                                    start=True, stop=True, skip_group_check=True)

        def absorb_pre(src_ap, one_ap, dty):
            return nc.tensor.transpose(sink.bitcast(dty)[0:1, 0:1], src_ap, one_ap)

        # ---------------- Phase 1: xT, T1T, gating ----------------
        with ExitStack() as s1:
            p1 = s1.enter_context(tc.tile_pool(name="p1", bufs=1))
            sm = s1.enter_context(tc.tile_pool(name="sm", bufs=2))
            ps_tp = s1.enter_context(tc.tile_pool(name="ps_tp", bufs=2, space="PSUM"))
            ps_t1 = s1.enter_context(tc.tile_pool(name="ps_t1", bufs=2, space="PSUM"))
            ps_lg = s1.enter_context(tc.tile_pool(name="ps_lg", bufs=1, space="PSUM"))
            ps_wb = s1.enter_context(tc.tile_pool(name="ps_wb", bufs=2, space="PSUM"))

            x_sb = p1.tile([128, TOKC, D], f32, tag="x")
            nc.sync.dma_start(x_sb, x_d.rearrange("(t p) d -> p t d", p=128))
            U1sb = p1.tile([128, DC, ER], f32r, tag="u1")
            nc.sync.dma_start(U1sb, u1c_d.rearrange("(dc p) er -> p dc er", p=128))
            xT = p1.tile([128, DC, NT], f32r, tag="xT")

            absorb(ident[0:1, 0:1])
            absorb(gw_sb[0:1, 0:1])
            absorb(x_sb[0:1, 0, 0:1])

            # gate_w.T tiles via PE transpose (evac: DVE)
            for dc in range(DC):
                pst = ps_tp.tile([128, 128], f32, tag="tp")
                nc.tensor.transpose(pst[:, 0:E], gw_sb[:, dc * 128:(dc + 1) * 128],
                                    ident[0:E, 0:E])
                nc.vector.tensor_copy(gwT[:, dc, :], pst[:, 0:E])

            # x.T tiles via PE transpose (evac: DVE)
            for dc in range(DC):
                for t in range(TOKC):
                    pst = ps_tp.tile([128, 128], f32, tag="tp")
                    nc.tensor.transpose(pst, x_sb[:, t, dc * 128:(dc + 1) * 128], ident)
                    nc.vector.tensor_copy(xT[:, dc, t * 128:(t + 1) * 128], pst)

            absorb(U1sb[0:1, 0, 0:1])

            # T1T = U1.T @ xT   [er, tok], accumulated over DC (evac: ACT)
            for c in range(ERC):
                pt = ps_t1.tile([128, NT], f32, tag="t1")
                for dc in range(DC):
                    nc.tensor.matmul(pt,
                                     lhsT=U1sb[:, dc, c * 128:(c + 1) * 128],
                                     rhs=xT[:, dc, :],
                                     start=(dc == 0), stop=(dc == DC - 1))
                nc.vector.tensor_copy(T1Tp[c], pt)

            absorb(gb_sb[0:1, 0:1])
            absorb(ones_sb[0:1, 0:1])

            # logits, top-2 renormalized weights
            for t in range(TOKC):
                pl = ps_lg.tile([128, E], f32, tag="lg")
                for dc in range(DC):
                    nc.tensor.matmul(pl,
                                     lhsT=xT[:, dc, t * 128:(t + 1) * 128],
                                     rhs=gwT[:, dc, :],
                                     start=(dc == 0), stop=False)
                nc.tensor.matmul(pl, lhsT=ones_sb, rhs=gb_sb,
                                 start=False, stop=True)
                lg = sm.tile([128, E], f32, tag="lg_sb")
                nc.vector.tensor_copy(lg, pl)
                l1 = sm.tile([128, 1], f32, tag="l1")
                nc.vector.reduce_max(out=l1, in_=lg, axis=AX.X)
                m1t = sm.tile([128, E], f32, tag="m1t")
                nc.vector.tensor_scalar(m1t, lg, l1, None, op0=ALU.is_equal)
                lm = sm.tile([128, E], f32, tag="lm")
                nc.vector.tensor_scalar(lm, m1t, -1e30, None, op0=ALU.mult)
                nc.vector.tensor_add(lm, lm, lg)
                l2 = sm.tile([128, 1], f32, tag="l2")
                nc.vector.reduce_max(out=l2, in_=lm, axis=AX.X)
                m2t = sm.tile([128, E], f32, tag="m2t")
                nc.vector.tensor_scalar(m2t, lm, l2, None, op0=ALU.is_equal)
                dif = sm.tile([128, 1], f32, tag="dif")
                nc.vector.tensor_sub(dif, l1, l2)
                s1v = sm.tile([128, 1], f32, tag="s1v")
                nc.scalar.activation(s1v, dif, AF.Sigmoid)
                s0v = sm.tile([128, 1], f32, tag="s0v")
                nc.scalar.activation(s0v, dif, AF.Sigmoid, scale=-1.0)
                wa = sm.tile([128, E], f32, tag="wa")
                nc.vector.tensor_scalar(wa, m1t, s1v, None, op0=ALU.mult)
                wb_ = sm.tile([128, E], f32, tag="wb_")
                nc.vector.tensor_scalar(wb_, m2t, s0v, None, op0=ALU.mult)
                w_sb = sm.tile([128, E], f32, tag="w_sb")
                nc.vector.tensor_add(w_sb, wa, wb_)
                pw = ps_tp.tile([128, 128], f32, tag="tp")
                nc.tensor.transpose(pw[0:E, :], w_sb, ident)
                nc.vector.tensor_copy(wT[:, t * 128:(t + 1) * 128], pw[0:E, :])

            absorb(sel_sb[0:1, 0, 0:1])

            # broadcast per-expert weights across partitions: SEL.T @ wT
            for c in range(ERC):
                pb = ps_wb.tile([128, NT], f32, tag="wb")
                nc.tensor.matmul(pb, lhsT=sel_sb[:, c, :], rhs=wT,
                                 start=True, stop=True)
                nc.vector.tensor_copy(Wbc[c], pb)

            # engine-tick observers: bring PE's view of the ACT/DVE clocks
            # current so later pool-alloc deps reduce to single waits
            obs_a = persist.tile([1, 1], f32, tag="obs_a")
            nc.scalar.copy(obs_a, b1r_sb[0:1, 0:1])
            absorb(obs_a)
            obs_late = absorb(Wbc[ERC - 1][0:1, 0:1])

        # ---------------- Phase 2: experts ----------------
        p2v = ctx.enter_context(tc.tile_pool(name="p2v", bufs=2))
        p2u = ctx.enter_context(tc.tile_pool(name="p2u", bufs=2))
        p2h = ctx.enter_context(tc.tile_pool(name="p2h", bufs=6))
        s2 = ExitStack()
        ps_h0 = s2.enter_context(tc.tile_pool(name="ps_h0", bufs=3, space="PSUM"))
        ps_h1 = s2.enter_context(tc.tile_pool(name="ps_h1", bufs=2, space="PSUM"))
        ps_t2 = s2.enter_context(tc.tile_pool(name="ps_t2", bufs=1, space="PSUM"))

        relu_i = 0

        def relu_evac(dst, src, bias_ap, force_dve=False):
            nonlocal relu_i
            relu_i += 1
            if not force_dve and relu_i % 16 < ACT_OF_16:
                nc.scalar.activation(dst, src, AF.Relu, bias=bias_ap)
            else:
                nc.vector.tensor_scalar(dst, src, bias_ap, 0.0,
                                        op0=ALU.add, op1=ALU.max)

        for c0 in range(ERC * rep):
            c = c0 % ERC
            e0, e1 = 2 * c, 2 * c + 1
            v1p = p2v.tile([128, H], f32r, tag="v1")
            absorb_pre(v1p.bitcast(bf16)[0:1, 0:1], one_bf, bf16)
            nc.sync.dma_start(
                v1p, v1_d[e0:e1 + 1].rearrange("two r h -> (two r) h"))
            u2p = p2u.tile([128, 2, HC, R], bf16, tag="u2")
            absorb_pre(u2p[0:1, 0, 0, 0:1], one_bf, bf16)
            nc.sync.dma_start(
                u2p, u2r_d[e0:e1 + 1].rearrange("two p hc r -> p two hc r"))

            if c0 == 1:  # phase-3 loads, emitted late so they don't hog queues
                for cc in range(ERC):
                    nc.sync.dma_start(V2sb[cc], v2c_d[cc * 128:(cc + 1) * 128, :])

            absorb(v1p[0:1, 0:1])
            absorb(u2p[0:1, 0, 0, 0:1])

            # separate PSUM banks per col-tile half (start=True is per-bank)
            pt2a = ps_t2.tile([128, NT], f32, tag="t2a", name="pt2a")
            pt2b = ps_t2.tile([128, NT], f32, tag="t2b", name="pt2b")
            if c0 == 0:
                # first PE touch of the ps_t2 pool soaks its alloc wait;
                # scheduling-only edge keeps it after the clock observers
                for pz in (pt2a, pt2b):
                    dmy = nc.tensor.matmul(pz[0:1, 0:1], lhsT=ident[0:1, 0:1],
                                           rhs=ident[0:1, 0:1], start=True,
                                           stop=True, skip_group_check=True)
                    add_dep_helper(dmy.ins, obs_late.ins, sync=False,
                                   reason="order pt2 pre-touch after observers")
            for hc in range(HC):
                ph0 = ps_h0.tile([128, NT], f32, tag="h0", name="ph0")
                ph1 = ps_h1.tile([128, NT], f32, tag="h1", name="ph1")
                # m2: row-tiled concurrent K=64 matmuls (one per expert)
                nc.tensor.matmul(ph0,
                                 lhsT=v1p[0:64, hc * 128:(hc + 1) * 128],
                                 rhs=T1Tp[c][0:64, :],
                                 start=True, stop=True)
                nc.tensor.matmul(ph1,
                                 lhsT=v1p[64:128, hc * 128:(hc + 1) * 128],
                                 rhs=T1Tp[c][64:128, :],
                                 start=True, stop=True)
                hT0 = p2h.tile([128, NT], bf16, tag="h0", name="hT0")
                hT1 = p2h.tile([128, NT], bf16, tag="h1", name="hT1")
                if c0 == 0 and hc == 0:
                    absorb_pre(hT0[0:1, 0:1], one_bf, bf16)
                    absorb_pre(hT1[0:1, 0:1], one_bf, bf16)
                relu_evac(hT0, ph0, b1r_sb[:, e0 * HC + hc:e0 * HC + hc + 1],
                          force_dve=(hc == 0))
                relu_evac(hT1, ph1, b1r_sb[:, e1 * HC + hc:e1 * HC + hc + 1],
                          force_dve=(hc == 0))
                # m3: col-tiled concurrent pair, bf16, accumulate over hc
                nc.tensor.matmul(pt2a[0:64, :], lhsT=u2p[:, 0, hc, :], rhs=hT0,
                                 start=(hc == 0), stop=(hc == HC - 1),
                                 skip_group_check=True)
                nc.tensor.matmul(pt2b[64:128, :], lhsT=u2p[:, 1, hc, :], rhs=hT1,
                                 start=(hc == 0), stop=(hc == HC - 1),
                                 skip_group_check=True)
            nc.vector.tensor_tensor(T2Ts[c][0:64, :], pt2a[0:64, :],
                                    Wbc[c][0:64, :], op=ALU.mult)
            nc.vector.tensor_tensor(T2Ts[c][64:128, :], pt2b[64:128, :],
                                    Wbc[c][64:128, :], op=ALU.mult)

        s2.close()

        # ---------------- Phase 3: combine ----------------
        absorb(b2_sb[0:1, 0:1])
        for c in range(ERC):
            absorb(V2sb[c][0:1, 0:1])
        absorb(T2Ts[ERC - 1][0:1, 0:1])
        ps_o = ctx.enter_context(tc.tile_pool(name="ps_o", bufs=2, space="PSUM"))
        p3o = ctx.enter_context(tc.tile_pool(name="p3o", bufs=4))
        for t in range(TOKC):
            for dd in range(DD):
                po = ps_o.tile([128, 512], f32, tag="o", name="po")
                for c in range(ERC):
                    nc.tensor.matmul(po,
                                     lhsT=T2Ts[c][:, t * 128:(t + 1) * 128],
                                     rhs=V2sb[c][:, dd * 512:(dd + 1) * 512],
                                     start=(c == 0), stop=False)
                nc.tensor.matmul(po, lhsT=wT[:, t * 128:(t + 1) * 128],
                                 rhs=b2_sb[:, dd * 512:(dd + 1) * 512],
                                 start=False, stop=True)
                ob = p3o.tile([128, 512], f32, tag="ob")
                nc.scalar.copy(ob, po)
                nc.sync.dma_start(
                    out_d[t * 128:(t + 1) * 128, dd * 512:(dd + 1) * 512], ob)

    nc.compile()
    return nc


def build_tiny():
    """Trivial NEFF for measuring dispatch overhead."""
    from contextlib import ExitStack as _ES
    nc = bacc.Bacc("TRN2", debug=False)
    a_d = nc.dram_tensor("a", [1, 8], dt.float32, kind="ExternalInput").ap()
    o_d = nc.dram_tensor("o", [1, 8], dt.float32, kind="ExternalOutput").ap()
    with _ES() as ctx:
        tc = ctx.enter_context(tile.TileContext(nc))
        p = ctx.enter_context(tc.tile_pool(name="p", bufs=1))
        t = p.tile([1, 8], dt.float32, tag="t")
        nc.sync.dma_start(t, a_d)
        nc.sync.dma_start(o_d, t)
    nc.compile()
    return nc


def prep_inputs(x, u1, v1, b1, u2, v2, b2, gate_w, gate_b, cfg):
    """Host-side layout prep (pure reshapes/transposes) + per-core sharding."""
    E, D, H, R, NT = cfg["E"], cfg["D"], cfg["H"], cfg["R"], cfg["NT"]
    HC = H // 128
    ER = E * R
    ERC = E // 2
    f = lambda a: np.ascontiguousarray(np.asarray(a, dtype=np.float32))

    x = f(x)
    u1c = f(np.asarray(u1, np.float32).transpose(1, 0, 2).reshape(D, ER))
    v1 = f(v1)
    # b1r[p, e*HC+hc] = b1[e, hc*128+p]  (per-partition bias columns)
    b1r = f(np.asarray(b1, np.float32).reshape(E, HC, 128).transpose(2, 0, 1)
            .reshape(128, E * HC))
    # u2r[e, p, hc, r] = u2[e, hc*128+p, r]
    import ml_dtypes
    u2r = np.ascontiguousarray(
        np.asarray(u2, np.float32).reshape(E, HC, 128, R).transpose(0, 2, 1, 3)
        .astype(ml_dtypes.bfloat16))
    v2c = f(np.asarray(v2, np.float32).reshape(ER, D))
    b2 = f(b2)
    gw = f(gate_w)
    gb = f(np.asarray(gate_b, np.float32).reshape(1, E))
    ident = np.eye(128, dtype=np.float32)
    onesr = np.ones((1, 128), dtype=np.float32)
    sel = np.zeros((E, ERC, 128), dtype=np.float32)
    for c in range(ERC):
        sel[2 * c, c, 0:64] = 1.0
        sel[2 * c + 1, c, 64:128] = 1.0

    ncores = x.shape[0] // NT
    shared = dict(u1c=u1c, v1=v1, b1r=b1r, u2r=u2r, v2c=v2c, b2=b2,
                  gw=gw, gb=gb, ident=ident, sel=sel, onesr=onesr)
    in_maps = []
    for c in range(ncores):
        m = dict(shared)
        m["x"] = np.ascontiguousarray(x[c * NT:(c + 1) * NT])
        in_maps.append(m)
    return in_maps


_BUILT = {}


def _get_nc(cfg_key=None):
    if cfg_key is None:
        cfg_key = "full"
    if cfg_key not in _BUILT:
        _BUILT[cfg_key] = build(FULL_CFG)
    return _BUILT[cfg_key]


def run(inputs, trace=False):
    """Run on 8 cores; returns (full_output, BassKernelResults)."""
    import concourse.bass_utils as bass_utils

    nc = _get_nc()
    in_maps = prep_inputs(**inputs, cfg=FULL_CFG)
    res = bass_utils.run_bass_kernel_spmd(
        nc, in_maps, core_ids=list(range(len(in_maps))), trace=trace)
    out = np.concatenate([r["out"] for r in res.results], axis=0)
    return out, res


def kernel(**inputs) -> np.ndarray:
    out, _ = run(inputs, trace=False)
    return out


if __name__ == "__main__":
    # smoke-build only
    nc = _get_nc()
    print("built ok:", nc)
